# revision 1
# baseline (speedup 1.0000x reference)
"""Deformable-DETR encoder (6 layers) on 8 trn2 NeuronCores.

Sharding: core c handles batch item b=c//2, query half h=c%2 (QH=2720
queries). On-chip state is feature-major ("transposed", [d, q]). Per layer
the value-projection halves are exchanged between the two cores of a pair
with an AllGather; everything else is local.

MSDeformAttn sampling: a bordered quad table T[(h,dh) partitions, qidx]
holds uint32 entries packing the (x0, x0+1) bf16 pair of one value row;
the row-above pair is the same table at qidx + (W_l+1). GPSIMD ap_gather
pulls both pairs per (query, head, level, point); bilinear+attention
weights, built in [(h,lp), q] layout and replicated across dh by PE
selector matmuls, multiply the gathered stream on DVE; a grouped
tensor_reduce sums the 32 (lp, corner) terms per query.
"""

import os
import numpy as np
import ml_dtypes

import concourse.bass as bass
import concourse.bacc as bacc
import concourse.mybir as mybir
import concourse.tile as tile
from concourse.bass_utils import run_bass_kernel_spmd

F32 = mybir.dt.float32
BF16 = mybir.dt.bfloat16
I16 = mybir.dt.int16
U32 = mybir.dt.uint32
AL = mybir.AluOpType
AF = mybir.ActivationFunctionType
AX = mybir.AxisListType

B, N, D, H, LVLS, PTS, DFF = 4, 5440, 256, 8, 4, 4, 1024
NLAYERS = int(os.environ.get("KERNEL_NLAYERS", "6"))
SHAPES = [(64, 64), (32, 32), (16, 16), (8, 8)]
LSTART = [0, 4096, 5120, 5376]
QH = 2720
MAGIC = 12582912.0  # 1.5*2^23 : (x+MAGIC)-MAGIC == round-to-nearest(x)

TDIM = [(h + 1, w + 1) for h, w in SHAPES]   # bordered quad grids
TSIZES = [a * b for a, b in TDIM]
TSTART = [0, 4225, 5314, 5603]
TTOT = 5684
VPAD = 66
VW = VPAD + N + 2

MMCH = [512] * 5 + [160]
GCH = [128] * 21 + [32]


def _chunks(sizes):
    off = 0
    for s in sizes:
        yield off, s
        off += s


def build_module(n_layers=NLAYERS):
    sim2 = bool(os.environ.get("KERNEL_SIM2"))
    ncore = 2 if sim2 else 8
    nc = bacc.Bacc("TRN2", target_bir_lowering=False, debug=False, num_devices=ncore)
    L = n_layers

    x0T = nc.dram_tensor("x0T", [2, 128, QH], F32, kind="ExternalInput")
    posT = nc.dram_tensor("posT", [2, 128, QH], F32, kind="ExternalInput")
    refx_d = nc.dram_tensor("refx", [128, QH], F32, kind="ExternalInput")
    refy_d = nc.dram_tensor("refy", [128, QH], F32, kind="ExternalInput")
    outT = nc.dram_tensor("outT", [2, 128, QH], F32, kind="ExternalOutput")
    Woffx_d = nc.dram_tensor("Woffx", [L, 2, 128, 128], BF16, kind="ExternalInput")
    Woffy_d = nc.dram_tensor("Woffy", [L, 2, 128, 128], BF16, kind="ExternalInput")
    Wattn_d = nc.dram_tensor("Wattn", [L, 2, 128, 128], BF16, kind="ExternalInput")
    Wval_d = nc.dram_tensor("Wval", [L, 2, 128, 256], BF16, kind="ExternalInput")
    Wout_d = nc.dram_tensor("Wout", [L, 2, 128, 256], BF16, kind="ExternalInput")
    W1_d = nc.dram_tensor("W1", [L, 2, 128, 1024], BF16, kind="ExternalInput")
    W2_d = nc.dram_tensor("W2", [L, 8, 128, 256], BF16, kind="ExternalInput")
    bias_all_d = nc.dram_tensor("bias_all", [L, 128, 25], F32, kind="ExternalInput")
    BIDX = {"boffx": 0, "boffy": 1, "battn": 2, "bval": 3, "bout": 5,
            "bl1": 7, "bl2": 15, "g1": 17, "be1": 19, "g2": 21, "be2": 23}
    sel_d = nc.dram_tensor("sel", [128, 32, 128], BF16, kind="ExternalInput")
    bones_d = nc.dram_tensor("bones", [128, 8], F32, kind="ExternalInput")
    sel16_d = nc.dram_tensor("sel16", [8, 128], F32, kind="ExternalInput")
    ones128_d = nc.dram_tensor("ones128", [128, 1], F32, kind="ExternalInput")
    ones1x_d = nc.dram_tensor("ones1x", [1, 128], F32, kind="ExternalInput")
    ccol_d = nc.dram_tensor("ccol", [128, 8], F32, kind="ExternalInput")
    # ccol: 0:W-1  1:W-2  2:H-1  3:H-2  4:W+1  5:tstart+W+2

    with tile.TileContext(nc) as tc:
        with (
            tc.tile_pool(name="const", bufs=1) as cpool,
            tc.tile_pool(name="wts", bufs=2) as wpool,
            tc.tile_pool(name="layer", bufs=1) as lpool,
            tc.tile_pool(name="tmp", bufs=2) as kpool,
            tc.tile_pool(name="gsb", bufs=2) as gpool,
            tc.tile_pool(name="dram", bufs=1, space="DRAM") as dpool,
        ):
            sel_t = cpool.tile([128, 32, 128], BF16, tag="sel", name="sel")
            nc.sync.dma_start(sel_t[:], sel_d[:])
            bones_t = cpool.tile([128, 8], F32, tag="bones", name="bones")
            nc.sync.dma_start(bones_t[:], bones_d[:])
            sel16_t = cpool.tile([8, 128], F32, tag="sel16", name="sel16")
            nc.sync.dma_start(sel16_t[:], sel16_d[:])
            ones128_t = cpool.tile([128, 1], F32, tag="o128", name="o128")
            nc.sync.dma_start(ones128_t[:], ones128_d[:])
            ones1x_t = cpool.tile([1, 128], F32, tag="o1x", name="o1x")
            nc.sync.dma_start(ones1x_t[:], ones1x_d[:])
            ccol = cpool.tile([128, 8], F32, tag="ccol", name="ccol")
            nc.sync.dma_start(ccol[:], ccol_d[:])

            def col(t, j):
                return t[:, j : j + 1]

            vfull = [cpool.tile([128, VW], BF16, tag=f"vfull{dt}", name=f"vfull{dt}") for dt in range(2)]
            for dt in range(2):
                nc.vector.memset(vfull[dt][:, 0:VPAD], 0.0)
                nc.vector.memset(vfull[dt][:, VPAD + N : VW], 0.0)

            cc_in = dpool.tile([256, QH], BF16)
            cc_out = dpool.tile([2, 256, QH], BF16)
            out_ping = dpool.tile([2, 128, QH], F32)
            out_pong = dpool.tile([2, 128, QH], F32)

            ntmp = [0]

            def T(shape=None, dtype=F32, grp="a"):
                ntmp[0] += 1
                tg = f"t{ntmp[0] % 8}"
                return kpool.tile(shape or [128, 512], dtype, tag=tg, name=tg)

            cur = x0T  # DRAM tensor holding current layer input (transposed)
            for li in range(n_layers):
                nxt = outT if li == n_layers - 1 else (out_ping if li % 2 == 0 else out_pong)

                Wval_t = [wpool.tile([128, 256], BF16, tag=f"wval{k}", name=f"wval{k}") for k in range(2)]
                Wout_t = [wpool.tile([128, 256], BF16, tag=f"wout{k}", name=f"wout{k}") for k in range(2)]
                Woffx_t = [wpool.tile([128, 128], BF16, tag=f"wofx{k}", name=f"wofx{k}") for k in range(2)]
                Woffy_t = [wpool.tile([128, 128], BF16, tag=f"wofy{k}", name=f"wofy{k}") for k in range(2)]
                Wattn_t = [wpool.tile([128, 128], BF16, tag=f"watn{k}", name=f"watn{k}") for k in range(2)]
                W1_t = [wpool.tile([128, 1024], BF16, tag=f"w1{k}", name=f"w1{k}") for k in range(2)]
                W2_t = [wpool.tile([128, 256], BF16, tag=f"w2{k}", name=f"w2{k}") for k in range(8)]
                for k in range(2):
                    nc.sync.dma_start(Wval_t[k][:], Wval_d[li, k])
                    nc.sync.dma_start(Wout_t[k][:], Wout_d[li, k])
                    nc.sync.dma_start(Woffx_t[k][:], Woffx_d[li, k])
                    nc.sync.dma_start(Woffy_t[k][:], Woffy_d[li, k])
                    nc.sync.dma_start(Wattn_t[k][:], Wattn_d[li, k])
                    nc.sync.dma_start(W1_t[k][:], W1_d[li, k])
                for k in range(8):
                    nc.sync.dma_start(W2_t[k][:], W2_d[li, k])
                ball = wpool.tile([128, 25], F32, tag="ball", name="ball")
                nc.sync.dma_start(ball[:], bias_all_d[li])

                def bcol(nm, k=0):
                    j = BIDX[nm] + k
                    return ball[:, j : j + 1]

                w4all = lpool.tile([128, 4, QH], BF16, tag="w4all", name="w4all")
                idxT = [lpool.tile([128, QH], I16, tag=f"idxT{j}", name=f"idxT{j}") for j in range(2)]

                # ---- fused S1+S3+S5 per chunk: value proj, offsets/attn,
                #      sampling weights, indices
                with tc.tile_pool(name=f"ps{li}", bufs=2, space="PSUM") as psp:
                    for co, cw in _chunks(MMCH):
                        qs = slice(co, co + cw)
                        och = [T(grp="o") for _ in range(2)]
                        qb = [T(dtype=BF16, grp="q") for _ in range(2)]
                        for k in range(2):
                            nc.sync.dma_start(och[k][:, :cw], cur[k, :, qs])
                            pc = T(grp="o")
                            nc.sync.dma_start(pc[:, :cw], posT[k, :, qs])
                            nc.vector.tensor_tensor(pc[:, :cw], och[k][:, :cw],
                                                    pc[:, :cw], AL.add)
                            nc.vector.tensor_copy(qb[k][:, :cw], pc[:, :cw])
                        # value projection -> cc_in (DRAM)
                        for dt in range(2):
                            ps = psp.tile([128, 512], F32, tag="mm", name="mm")
                            ob = [T(dtype=BF16, grp="q") for _ in range(2)]
                            for k in range(2):
                                nc.vector.tensor_copy(ob[k][:, :cw], och[k][:, :cw])
                            for k in range(2):
                                nc.tensor.matmul(
                                    ps[:, :cw], Wval_t[k][:, dt * 128 : dt * 128 + 128],
                                    ob[k][:, :cw], start=(k == 0), stop=(k == 1))
                            vch = T(dtype=BF16, grp="v")
                            nc.scalar.activation(vch[:, :cw], ps[:, :cw], AF.Identity,
                                                 bias=bcol("bval", dt))
                            nc.sync.dma_start(cc_in[dt * 128 : dt * 128 + 128, qs],
                                              vch[:, :cw])

                        def proj128(wt, bcol):
                            ps = psp.tile([128, 512], F32, tag="mm", name="mm")
                            for k in range(2):
                                nc.tensor.matmul(ps[:, :cw], wt[k][:], qb[k][:, :cw],
                                                 start=(k == 0), stop=(k == 1))
                            o = T(grp="p")
                            nc.scalar.activation(o[:, :cw], ps[:, :cw], AF.Identity,
                                                 bias=bcol)
                            return o

                        offx = proj128(Woffx_t, bcol("boffx", 0))
                        offy = proj128(Woffy_t, bcol("boffy", 0))
                        psl = psp.tile([128, 512], F32, tag="mm", name="mm")
                        for k in range(2):
                            nc.tensor.matmul(psl[:, :cw], Wattn_t[k][:], qb[k][:, :cw],
                                             start=(k == 0), stop=(k == 1))
                        expt = T(grp="p")
                        nc.scalar.activation(expt[:, :cw], psl[:, :cw], AF.Exp,
                                             bias=bcol("battn", 0))
                        psd = psp.tile([8, 512], F32, tag="den", name="den")
                        nc.tensor.matmul(psd[:, :cw], bones_t[:], expt[:, :cw])
                        r8 = T([8, 512], grp="r")
                        nc.vector.reciprocal(r8[:, :cw], psd[:, :cw])
                        psr = psp.tile([128, 512], F32, tag="rep", name="rep")
                        nc.tensor.matmul(psr[:, :cw], sel16_t[:], r8[:, :cw])
                        attn = T(grp="p")
                        nc.vector.tensor_tensor(attn[:, :cw], expt[:, :cw],
                                                psr[:, :cw], AL.mult)

                        def floorfrac(off_sb, ref_dram):
                            x = T(grp="c")
                            rc = T(grp="c")
                            nc.sync.dma_start(rc[:, :cw], ref_dram[:, qs])
                            nc.vector.tensor_tensor(x[:, :cw], off_sb[:, :cw],
                                                    rc[:, :cw], AL.add)
                            r = T(grp="c")
                            nc.vector.tensor_scalar_add(r[:, :cw], x[:, :cw], MAGIC)
                            nc.vector.tensor_scalar_sub(r[:, :cw], r[:, :cw], MAGIC)
                            m = T(grp="c")
                            nc.vector.tensor_tensor(m[:, :cw], r[:, :cw], x[:, :cw],
                                                    AL.is_gt)
                            x0 = T(grp="f")
                            nc.vector.tensor_tensor(x0[:, :cw], r[:, :cw], m[:, :cw],
                                                    AL.subtract)
                            fx = T(grp="f")
                            nc.vector.tensor_tensor(fx[:, :cw], x[:, :cw], x0[:, :cw],
                                                    AL.subtract)
                            return x0, fx

                        x0, fx = floorfrac(offx, refx_d)
                        y0, fy = floorfrac(offy, refy_d)

                        def uv(c0, frac, hij):
                            a = T(grp="u")
                            nc.vector.tensor_scalar(a[:, :cw], c0[:, :cw], 0.0, None,
                                                    AL.is_ge)
                            b = T(grp="u")
                            nc.vector.tensor_scalar(b[:, :cw], c0[:, :cw],
                                                    col(ccol, hij), None, AL.is_le)
                            nc.vector.tensor_tensor(a[:, :cw], a[:, :cw], b[:, :cw],
                                                    AL.mult)
                            a1 = T(grp="u")
                            nc.vector.tensor_scalar(a1[:, :cw], c0[:, :cw], -1.0, None,
                                                    AL.is_ge)
                            b1 = T(grp="u")
                            nc.vector.tensor_scalar(b1[:, :cw], c0[:, :cw],
                                                    col(ccol, hij + 1), None, AL.is_le)
                            nc.vector.tensor_tensor(a1[:, :cw], a1[:, :cw], b1[:, :cw],
                                                    AL.mult)
                            omf = T(grp="w")
                            nc.vector.tensor_scalar(omf[:, :cw], frac[:, :cw], -1.0,
                                                    1.0, AL.mult, AL.add)
                            u0 = T(grp="w")
                            nc.vector.tensor_tensor(u0[:, :cw], omf[:, :cw], a[:, :cw],
                                                    AL.mult)
                            u1 = T(grp="w")
                            nc.vector.tensor_tensor(u1[:, :cw], frac[:, :cw],
                                                    a1[:, :cw], AL.mult)
                            return u0, u1

                        ux0, ux1 = uv(x0, fx, 0)
                        ty0, ty1 = uv(y0, fy, 2)
                        at0 = T(grp="w")
                        nc.vector.tensor_tensor(at0[:, :cw], attn[:, :cw], ty0[:, :cw],
                                                AL.mult)
                        at1 = T(grp="w")
                        nc.vector.tensor_tensor(at1[:, :cw], attn[:, :cw], ty1[:, :cw],
                                                AL.mult)
                        nc.vector.tensor_tensor(w4all[:, 0, qs], at0[:, :cw],
                                                ux0[:, :cw], AL.mult)
                        nc.vector.tensor_tensor(w4all[:, 1, qs], at0[:, :cw],
                                                ux1[:, :cw], AL.mult)
                        nc.vector.tensor_tensor(w4all[:, 2, qs], at1[:, :cw],
                                                ux0[:, :cw], AL.mult)
                        nc.vector.tensor_tensor(w4all[:, 3, qs], at1[:, :cw],
                                                ux1[:, :cw], AL.mult)
                        cx = T(grp="i")
                        nc.vector.tensor_scalar_max(cx[:, :cw], x0[:, :cw], -1.0)
                        nc.vector.tensor_scalar(cx[:, :cw], cx[:, :cw], col(ccol, 0),
                                                None, AL.min)
                        cy = T(grp="i")
                        nc.vector.tensor_scalar_max(cy[:, :cw], y0[:, :cw], -1.0)
                        nc.vector.tensor_scalar(cy[:, :cw], cy[:, :cw], col(ccol, 2),
                                                None, AL.min)
                        qi = T(grp="i")
                        nc.vector.tensor_scalar(qi[:, :cw], cy[:, :cw], col(ccol, 4),
                                                col(ccol, 5), AL.mult, AL.add)
                        nc.vector.tensor_tensor(qi[:, :cw], qi[:, :cw], cx[:, :cw],
                                                AL.add)
                        nc.vector.tensor_copy(idxT[0][:, qs], qi[:, :cw])
                        nc.vector.tensor_scalar(qi[:, :cw], qi[:, :cw], col(ccol, 4),
                                                None, AL.add)
                        nc.vector.tensor_scalar(qi[:, :cw], qi[:, :cw],
                                                float(TTOT - 1), None, AL.min)
                        nc.vector.tensor_copy(idxT[1][:, qs], qi[:, :cw])

                # ---- exchange value halves
                nc.gpsimd.collective_compute(
                    "AllGather", AL.bypass,
                    replica_groups=[[0, 1]] if sim2 else [[0, 1], [2, 3], [4, 5], [6, 7]],
                    ins=[cc_in[:].opt()], outs=[cc_out[:].opt()])
                for r in range(2):
                    for dt in range(2):
                        nc.sync.dma_start(
                            vfull[dt][:, VPAD + r * QH : VPAD + (r + 1) * QH],
                            cc_out[r, dt * 128 : dt * 128 + 128, :])

                # ---- gather + blend per head-half
                msdaT = [lpool.tile([128, QH], BF16, tag=f"msdaT{hh}", name=f"msdaT{hh}") for hh in range(2)]
                for hh in range(2):
                    tb = lpool.tile([128, TTOT + 1], U32, tag="quadtab", name="quadtab")
                    tbv = tb[:].bitcast(BF16)
                    for lv in range(LVLS):
                        th, tw = TDIM[lv]
                        Ww = SHAPES[lv][1]
                        for j in range(2):
                            sbase = VPAD + LSTART[lv] - Ww - 1 + j
                            vb = vfull[hh][:]
                            src3 = bass.AP(
                                vb.tensor, vb.offset + sbase,
                                [list(vb.ap[0]), [Ww, th], [1, tw]])
                            dbase = 2 * TSTART[lv] + j
                            dst3 = bass.AP(
                                tbv.tensor, tbv.offset + dbase,
                                [list(tbv.ap[0]), [2 * tw, th], [2, tw]])
                            nc.scalar.copy(dst3, src3)
                    idxs = [lpool.tile([128, QH], I16, tag=f"idxs{j}", name=f"idxs{j}") for j in range(2)]
                    for j in range(2):
                        for h4 in range(4):
                            srows = (4 * hh + h4) * 16
                            for dl in range(2):
                                drows = (2 * h4 + dl) * 16
                                nc.sync.dma_start(
                                    idxs[j][drows : drows + 16, :],
                                    idxT[j][srows : srows + 16, :])
                    with tc.tile_pool(name=f"psw{li}_{hh}", bufs=1,
                                      space="PSUM") as pswp:
                        for co, cw in _chunks(GCH):
                            nidx = cw * 16
                            w4ps = pswp.tile([128, 8, 512], F32, tag="w4ps", name="w4ps")
                            w4rep = gpool.tile([128, 2, 128, 16, 2], BF16, tag="w4rep", bufs=1, name="w4rep")
                            for rr in range(2):
                                for l8 in range(8):
                                    lp = rr * 8 + l8
                                    nc.tensor.matmul(
                                        w4ps[:, l8, : 4 * cw],
                                        sel_t[:, hh * 16 + lp, :],
                                        w4all[:, :, co : co + cw])
                                for pg in range(2):
                                    src = w4ps[:, :, pg * 2 * cw : (pg + 1) * 2 * cw]
                                    src4 = src.rearrange("p l (s q) -> p l s q", s=2)
                                    dst4 = w4rep[:, pg, :cw, rr * 8 : rr * 8 + 8, :]\
                                        .rearrange("p q l s -> p l s q")
                                    nc.scalar.copy(dst4, src4)
                            pt = []
                            for pg in range(2):
                                g = gpool.tile([128, 2048], U32, tag="G", name="G")
                                nc.gpsimd.ap_gather(
                                    g[:, :nidx], tb[:, :TTOT],
                                    idxs[pg][:, co : co + cw],
                                    channels=128, num_elems=TTOT, d=1, num_idxs=nidx)
                                gv = g[:, :nidx].bitcast(BF16)
                                w4flat = w4rep[:, pg, :cw, :, :].rearrange(
                                    "p q l s -> p (q l s)")
                                nc.vector.tensor_tensor(gv, gv, w4flat, AL.mult)
                                p_ = gpool.tile([128, 128], F32, tag=f"part{pg}", name=f"part{pg}")
                                nc.vector.tensor_reduce(
                                    p_[:, :cw],
                                    gv.rearrange("p (q k) -> p q k", k=32),
                                    AX.X, AL.add, opt_input=False)
                                pt.append(p_)
                            nc.vector.tensor_tensor(
                                msdaT[hh][:, co : co + cw], pt[0][:, :cw],
                                pt[1][:, :cw], AL.add)

                # ---- W_out + residual + LN1 ; FFN + residual + LN2
                with tc.tile_pool(name=f"pso{li}", bufs=2, space="PSUM") as psp:
                    for co, cw in _chunks(MMCH):
                        qs = slice(co, co + cw)

                        def layernorm(xin, gname, bename, dst0, dst1, outdram):
                            pss = psp.tile([1, 512], F32, tag="st1", bufs=1, name="st1")
                            for k in range(2):
                                nc.tensor.matmul(pss[:, :cw], ones128_t[:],
                                                 xin[k][:, :cw],
                                                 start=(k == 0), stop=(k == 1))
                            psq = psp.tile([1, 512], F32, tag="st2", bufs=1, name="st2")
                            for k in range(2):
                                xsq = T(grp="s")
                                nc.vector.tensor_tensor(xsq[:, :cw], xin[k][:, :cw],
                                                        xin[k][:, :cw], AL.mult)
                                nc.tensor.matmul(psq[:, :cw], ones128_t[:],
                                                 xsq[:, :cw],
                                                 start=(k == 0), stop=(k == 1))
                            mu = T([1, 512], grp="m")
                            nc.vector.tensor_scalar_mul(mu[:, :cw], pss[:, :cw],
                                                        1.0 / D)
                            var = T([1, 512], grp="m")
                            nc.vector.tensor_scalar_mul(var[:, :cw], psq[:, :cw],
                                                        1.0 / D)
                            mu2 = T([1, 512], grp="m")
                            nc.vector.tensor_tensor(mu2[:, :cw], mu[:, :cw],
                                                    mu[:, :cw], AL.mult)
                            nc.vector.tensor_tensor(var[:, :cw], var[:, :cw],
                                                    mu2[:, :cw], AL.subtract)
                            nc.vector.tensor_scalar_add(var[:, :cw], var[:, :cw], 1e-5)
                            rv = T([1, 512], grp="m")
                            nc.vector.reciprocal(rv[:, :cw], var[:, :cw])
                            rstd = T([1, 512], grp="m")
                            nc.scalar.activation(rstd[:, :cw], rv[:, :cw], AF.Sqrt)
                            psmu = psp.tile([128, 512], F32, tag="rpm", bufs=1, name="rpm")
                            nc.tensor.matmul(psmu[:, :cw], ones1x_t[:], mu[:, :cw])
                            psrs = psp.tile([128, 512], F32, tag="rps", bufs=1, name="rps")
                            nc.tensor.matmul(psrs[:, :cw], ones1x_t[:], rstd[:, :cw])
                            for k, dst in enumerate([dst0, dst1]):
                                xc = T(grp="s")
                                nc.vector.tensor_tensor(xc[:, :cw], xin[k][:, :cw],
                                                        psmu[:, :cw], AL.subtract)
                                nc.vector.tensor_tensor(xc[:, :cw], xc[:, :cw],
                                                        psrs[:, :cw], AL.mult)
                                nc.scalar.activation(dst[:, :cw], xc[:, :cw],
                                                     AF.Identity,
                                                     scale=bcol(gname, k),
                                                     bias=bcol(bename, k))
                                if outdram is not None:
                                    nc.sync.dma_start(outdram[k, :, qs], dst[:, :cw])

                        x1 = []
                        for dt in range(2):
                            ps = psp.tile([128, 512], F32, tag="mm", name="mm")
                            for k in range(2):
                                nc.tensor.matmul(
                                    ps[:, :cw],
                                    Wout_t[k][:, dt * 128 : dt * 128 + 128],
                                    msdaT[k][:, qs], start=(k == 0), stop=(k == 1))
                            t0 = T(grp="x")
                            nc.scalar.activation(t0[:, :cw], ps[:, :cw], AF.Identity,
                                                 bias=bcol("bout", dt))
                            och = T(grp="x")
                            nc.sync.dma_start(och[:, :cw], cur[dt, :, qs])
                            nc.vector.tensor_tensor(t0[:, :cw], t0[:, :cw],
                                                    och[:, :cw], AL.add)
                            x1.append(t0)
                        ln1 = [T(grp="l") for _ in range(2)]
                        layernorm(x1, "g1", "be1", ln1[0], ln1[1], None)
                        ln1b = [T(dtype=BF16, grp="lb") for _ in range(2)]
                        for dt in range(2):
                            nc.vector.tensor_copy(ln1b[dt][:, :cw], ln1[dt][:, :cw])
                        hidb = [T(dtype=BF16, grp=f"h{m}") for m in range(8)]
                        for m in range(8):
                            ph = psp.tile([128, 512], F32, tag="mm", name="mm")
                            for k in range(2):
                                nc.tensor.matmul(
                                    ph[:, :cw], W1_t[k][:, m * 128 : m * 128 + 128],
                                    ln1b[k][:, :cw], start=(k == 0), stop=(k == 1))
                            nc.scalar.activation(hidb[m][:, :cw], ph[:, :cw], AF.Relu,
                                                 bias=bcol("bl1", m))
                        x2 = []
                        for dt in range(2):
                            ps = psp.tile([128, 512], F32, tag="mm", name="mm")
                            for k in range(8):
                                nc.tensor.matmul(
                                    ps[:, :cw],
                                    W2_t[k][:, dt * 128 : dt * 128 + 128],
                                    hidb[k][:, :cw], start=(k == 0), stop=(k == 7))
                            t0 = T(grp="x")
                            nc.scalar.activation(t0[:, :cw], ps[:, :cw], AF.Identity,
                                                 bias=bcol("bl2", dt))
                            nc.vector.tensor_tensor(t0[:, :cw], t0[:, :cw],
                                                    ln1[dt][:, :cw], AL.add)
                            x2.append(t0)
                        no = [T(grp="n") for _ in range(2)]
                        layernorm(x2, "g2", "be2", no[0], no[1], nxt)
                cur = nxt

    nc.compile()
    return nc


# ---------------- host side ----------------

def _host_prep(inputs, n_layers=NLAYERS):
    f32 = np.float32
    bf16 = ml_dtypes.bfloat16
    L = n_layers
    inputs = dict(inputs)
    for nm in ["W_off", "b_off", "W_attn", "b_attn", "W_val", "b_val",
               "W_out", "b_out", "W1", "bl1", "W2", "bl2",
               "g1", "be1", "g2", "be2"]:
        inputs[nm] = np.asarray(inputs[nm])[:L]
    vr = np.asarray(inputs["valid_ratios"], f32)
    refs = []
    for lvl, (H_, W_) in enumerate(SHAPES):
        ry, rx = np.meshgrid(
            np.linspace(0.5, H_ - 0.5, H_, dtype=f32),
            np.linspace(0.5, W_ - 0.5, W_, dtype=f32), indexing="ij")
        ry = ry.reshape(-1)[None] / (vr[:, None, lvl, 1] * H_)
        rx = rx.reshape(-1)[None] / (vr[:, None, lvl, 0] * W_)
        refs.append(np.stack([rx, ry], -1))
    ref = np.concatenate(refs, 1)
    ref = ref[:, :, None] * vr[:, None]                    # [B, N, LVLS, 2]

    Wd = np.array([w for h, w in SHAPES], f32)
    Hd = np.array([h for h, w in SHAPES], f32)
    lrow = np.tile(np.repeat(np.arange(LVLS), PTS), H)     # [128]
    refx_all = ref[:, :, :, 0] * Wd[None, None] - 0.5
    refy_all = ref[:, :, :, 1] * Hd[None, None] - 0.5

    W_off = np.asarray(inputs["W_off"], f32).reshape(L, D, H, LVLS, PTS, 2)
    b_off = np.asarray(inputs["b_off"], f32).reshape(L, H, LVLS, PTS, 2)
    Woffx = W_off[..., 0].reshape(L, D, 128)
    Woffy = W_off[..., 1].reshape(L, D, 128)

    def kt(w, nk):
        return np.ascontiguousarray(
            np.asarray(w, f32).reshape(L, nk, 128, -1)).astype(bf16)

    def bc(v, w):
        return np.ascontiguousarray(
            np.asarray(v, f32).reshape(L, w, 128).transpose(0, 2, 1))

    shared = {
        "Woffx": kt(Woffx, 2), "Woffy": kt(Woffy, 2),
        "Wattn": kt(inputs["W_attn"], 2), "Wval": kt(inputs["W_val"], 2),
        "Wout": kt(inputs["W_out"], 2), "W1": kt(inputs["W1"], 2),
        "W2": kt(inputs["W2"], 8),
    }
    bias_all = np.zeros((L, 128, 25), f32)
    bias_all[:, :, 0] = b_off[..., 0].reshape(L, 128)
    bias_all[:, :, 1] = b_off[..., 1].reshape(L, 128)
    bias_all[:, :, 2] = np.asarray(inputs["b_attn"], f32).reshape(L, 128)
    bias_all[:, :, 3:5] = bc(inputs["b_val"], 2)
    bias_all[:, :, 5:7] = bc(inputs["b_out"], 2)
    bias_all[:, :, 7:15] = bc(inputs["bl1"], 8)
    bias_all[:, :, 15:17] = bc(inputs["bl2"], 2)
    bias_all[:, :, 17:19] = bc(inputs["g1"], 2)
    bias_all[:, :, 19:21] = bc(inputs["be1"], 2)
    bias_all[:, :, 21:23] = bc(inputs["g2"], 2)
    bias_all[:, :, 23:25] = bc(inputs["be2"], 2)
    shared["bias_all"] = bias_all
    sel = np.zeros((128, 32, 128), f32)
    for hh in range(2):
        for lp in range(16):
            for h4 in range(4):
                sel[(4 * hh + h4) * 16 + lp, hh * 16 + lp,
                    h4 * 32 : h4 * 32 + 32] = 1.0
    shared["sel"] = sel.astype(bf16)
    bones = np.zeros((128, 8), f32)
    for h in range(H):
        bones[h * 16 : h * 16 + 16, h] = 1.0
    shared["bones"] = bones
    sel16 = np.zeros((8, 128), f32)
    for h in range(H):
        sel16[h, h * 16 : h * 16 + 16] = 1.0
    shared["sel16"] = sel16
    shared["ones128"] = np.ones((128, 1), f32)
    shared["ones1x"] = np.ones((1, 128), f32)
    ccol = np.zeros((128, 8), f32)
    for p in range(128):
        lv = lrow[p]
        ccol[p, 0] = Wd[lv] - 1
        ccol[p, 1] = Wd[lv] - 2
        ccol[p, 2] = Hd[lv] - 1
        ccol[p, 3] = Hd[lv] - 2
        ccol[p, 4] = Wd[lv] + 1
        ccol[p, 5] = TSTART[lv] + Wd[lv] + 2
    shared["ccol"] = ccol

    src = np.asarray(inputs["src"], f32)
    pos = np.asarray(inputs["pos"], f32)
    per_core = []
    for c in range(8):
        b, hf = c // 2, c % 2
        qs = slice(hf * QH, (hf + 1) * QH)
        m = dict(shared)
        m["x0T"] = np.ascontiguousarray(src[b, qs].T).reshape(2, 128, QH)
        m["posT"] = np.ascontiguousarray(pos[b, qs].T).reshape(2, 128, QH)
        m["refx"] = np.ascontiguousarray(refx_all[b, qs][:, lrow].T)
        m["refy"] = np.ascontiguousarray(refy_all[b, qs][:, lrow].T)
        per_core.append(m)
    return per_core


_NC_CACHE = {}


def kernel(**inputs):
    if NLAYERS not in _NC_CACHE:
        _NC_CACHE[NLAYERS] = build_module(NLAYERS)
    nc = _NC_CACHE[NLAYERS]
    in_maps = _host_prep(inputs, NLAYERS)
    res = run_bass_kernel_spmd(nc, in_maps, core_ids=list(range(8)))
    out = np.empty((B, N, D), np.float32)
    for c in range(8):
        b, hf = c // 2, c % 2
        o = res.results[c]["outT"]
        out[b, hf * QH : (hf + 1) * QH, :] = o.reshape(256, QH).T
    return out


if __name__ == "__main__":
    import reference
    inp = {k: np.asarray(v) for k, v in reference.setup_inputs().items()}
    got = kernel(**inp)
    print("kernel output:", got.shape, got.dtype)



# revision 8
# speedup vs baseline: 554.3259x; 554.3259x over previous
"""Deformable-DETR encoder (6 layers) on 8 trn2 NeuronCores.

Sharding: core c handles batch item b=c//2, query half h=c%2 (QH=2720
queries). On-chip state is feature-major ("transposed", [d, q]). Per layer
the value-projection halves are exchanged between the two cores of a pair
with an AllGather; everything else is local.

MSDeformAttn sampling: a quad table T[(h,dp) partitions, t, 4xu32] packs,
per bordered-grid position t, the 2x2 bilinear corner pairs for TWO value
channels (dh=2dp, 2dp+1) as 8 bf16. One GPSIMD ap_gather index (d=4)
fetches all four corners for both channels of every head (core k = head k),
so the index stream is 4x shorter than a per-corner gather. The bilinear+
attention corner weights, replicated across dh by PE selector matmuls into
PSUM, multiply the gathered stream on DVE (one pass per dh parity); two
strided XY tensor_reduces produce the parity-interleaved MSDA output.
Host-side permutations of W_val/b_val columns and W_out contract rows make
the (head, dh-pair, parity) channel layout free.
"""

import os
import numpy as np
import ml_dtypes

import concourse.bass as bass
import concourse.bacc as bacc
import concourse.mybir as mybir
import concourse.tile as tile
from concourse.bass_utils import run_bass_kernel_spmd

F32 = mybir.dt.float32
BF16 = mybir.dt.bfloat16
I16 = mybir.dt.int16
U32 = mybir.dt.uint32
AL = mybir.AluOpType
AF = mybir.ActivationFunctionType
AX = mybir.AxisListType

B, N, D, H, LVLS, PTS, DFF = 4, 5440, 256, 8, 4, 4, 1024
NLAYERS = int(os.environ.get("KERNEL_NLAYERS", "6"))
SHAPES = [(64, 64), (32, 32), (16, 16), (8, 8)]
LSTART = [0, 4096, 5120, 5376]
QH = 2720
MAGIC = 12582912.0  # 1.5*2^23 : (x+MAGIC)-MAGIC == round-to-nearest(x)

TDIM = [(h + 1, w + 1) for h, w in SHAPES]   # bordered quad grids
TSIZES = [a * b for a, b in TDIM]
TSTART = [0, 4225, 5314, 5603]
TTOT = 5684
VPAD = 66
EPAD = 16
VW2 = VPAD + N + EPAD

MMCH = [512] * 5 + [160]
BCH = [64] * 42 + [32]  # blend chunk widths (queries per gather)

def _chunks(sizes):
    off = 0
    for s in sizes:
        yield off, s
        off += s


def build_module(n_layers=NLAYERS):
    sim2 = bool(os.environ.get("KERNEL_SIM2"))
    ncore = 2 if sim2 else 8
    nc = bacc.Bacc("TRN2", target_bir_lowering=False, debug=False, num_devices=ncore)
    L = n_layers

    x0T = nc.dram_tensor("x0T", [2, 128, QH], F32, kind="ExternalInput")
    posT = nc.dram_tensor("posT", [2, 128, QH], F32, kind="ExternalInput")
    refx_d = nc.dram_tensor("refx", [128, QH], F32, kind="ExternalInput")
    refy_d = nc.dram_tensor("refy", [128, QH], F32, kind="ExternalInput")
    outT = nc.dram_tensor("outT", [2, 128, QH], F32, kind="ExternalOutput")
    Woffx_d = nc.dram_tensor("Woffx", [L, 2, 128, 128], BF16, kind="ExternalInput")
    Woffy_d = nc.dram_tensor("Woffy", [L, 2, 128, 128], BF16, kind="ExternalInput")
    Wattn_d = nc.dram_tensor("Wattn", [L, 2, 128, 128], BF16, kind="ExternalInput")
    Wval_d = nc.dram_tensor("Wval", [L, 2, 128, 256], BF16, kind="ExternalInput")
    Wout_d = nc.dram_tensor("Wout", [L, 2, 128, 256], BF16, kind="ExternalInput")
    W1_d = nc.dram_tensor("W1", [L, 2, 128, 1024], BF16, kind="ExternalInput")
    W2_d = nc.dram_tensor("W2", [L, 8, 128, 256], BF16, kind="ExternalInput")
    bias_all_d = nc.dram_tensor("bias_all", [L, 128, 25], F32, kind="ExternalInput")
    BIDX = {"boffx": 0, "boffy": 1, "battn": 2, "bval": 3, "bout": 5,
            "bl1": 7, "bl2": 15, "g1": 17, "be1": 19, "g2": 21, "be2": 23}
    sel_d = nc.dram_tensor("sel", [128, 16, 128], BF16, kind="ExternalInput")
    bones_d = nc.dram_tensor("bones", [128, 8], F32, kind="ExternalInput")
    sel16_d = nc.dram_tensor("sel16", [8, 128], F32, kind="ExternalInput")
    ones128_d = nc.dram_tensor("ones128", [128, 1], F32, kind="ExternalInput")
    ones1x_d = nc.dram_tensor("ones1x", [1, 128], F32, kind="ExternalInput")
    ccol_d = nc.dram_tensor("ccol", [128, 8], F32, kind="ExternalInput")
    # ccol: 0:W-1  1:W-2  2:H-1  3:H-2  4:W+1  5:tstart+W+2

    with tile.TileContext(nc) as tc:
        with (
            tc.tile_pool(name="const", bufs=1) as cpool,
            tc.tile_pool(name="wts", bufs=1) as wpool,
            tc.tile_pool(name="layer", bufs=1) as lpool,
            tc.tile_pool(name="dram", bufs=1, space="DRAM") as dpool,
        ):
            sel_t = cpool.tile([128, 16, 128], BF16, tag="sel", name="sel")
            nc.sync.dma_start(sel_t[:], sel_d[:])
            bones_t = cpool.tile([128, 8], F32, tag="bones", name="bones")
            nc.sync.dma_start(bones_t[:], bones_d[:])
            sel16_t = cpool.tile([8, 128], F32, tag="sel16", name="sel16")
            nc.sync.dma_start(sel16_t[:], sel16_d[:])
            ones128_t = cpool.tile([128, 1], F32, tag="o128", name="o128")
            nc.sync.dma_start(ones128_t[:], ones128_d[:])
            ones1x_t = cpool.tile([1, 128], F32, tag="o1x", name="o1x")
            nc.sync.dma_start(ones1x_t[:], ones1x_d[:])
            ccol = cpool.tile([128, 8], F32, tag="ccol", name="ccol")
            nc.sync.dma_start(ccol[:], ccol_d[:])

            def col(t, j):
                return t[:, j : j + 1]

            cc_in = dpool.tile([256, QH], BF16)
            cc_out = dpool.tile([2, 256, QH], BF16)
            out_ping = dpool.tile([2, 128, QH], F32)
            out_pong = dpool.tile([2, 128, QH], F32)

            ntmp = [0]
            cur_kpool = [None]

            def T(shape=None, dtype=F32, grp="a"):
                ntmp[0] += 1
                tg = f"t{ntmp[0] % 8}"
                return cur_kpool[0].tile(shape or [128, 512], dtype, tag=tg, name=tg)

            cur = x0T  # DRAM tensor holding current layer input (transposed)
            for li in range(n_layers):
                nxt = outT if li == n_layers - 1 else (out_ping if li % 2 == 0 else out_pong)

                Wval_t = [wpool.tile([128, 256], BF16, tag=f"wval{k}", name=f"wval{k}") for k in range(2)]
                Wout_t = [wpool.tile([128, 256], BF16, tag=f"wout{k}", name=f"wout{k}") for k in range(2)]
                Woffx_t = [wpool.tile([128, 128], BF16, tag=f"wofx{k}", name=f"wofx{k}") for k in range(2)]
                Woffy_t = [wpool.tile([128, 128], BF16, tag=f"wofy{k}", name=f"wofy{k}") for k in range(2)]
                Wattn_t = [wpool.tile([128, 128], BF16, tag=f"watn{k}", name=f"watn{k}") for k in range(2)]
                W1_t = [wpool.tile([128, 1024], BF16, tag=f"w1{k}", name=f"w1{k}") for k in range(2)]
                W2_t = [wpool.tile([128, 256], BF16, tag=f"w2{k}", name=f"w2{k}") for k in range(8)]
                for k in range(2):
                    nc.sync.dma_start(Wval_t[k][:], Wval_d[li, k])
                    nc.sync.dma_start(Wout_t[k][:], Wout_d[li, k])
                    nc.sync.dma_start(Woffx_t[k][:], Woffx_d[li, k])
                    nc.sync.dma_start(Woffy_t[k][:], Woffy_d[li, k])
                    nc.sync.dma_start(Wattn_t[k][:], Wattn_d[li, k])
                    nc.sync.dma_start(W1_t[k][:], W1_d[li, k])
                for k in range(8):
                    nc.sync.dma_start(W2_t[k][:], W2_d[li, k])
                ball = wpool.tile([128, 25], F32, tag="ball", name="ball")
                nc.sync.dma_start(ball[:], bias_all_d[li])

                def bcol(nm, k=0):
                    j = BIDX[nm] + k
                    return ball[:, j : j + 1]

                w4all = lpool.tile([128, 4, QH], BF16, tag="w4all", name="w4all")
                idxT = lpool.tile([128, QH], I16, tag="idxT", name="idxT")
                msdaF = lpool.tile([128, 2 * QH], F32, tag="msdaF", name="msdaF")

                # ---- fused S1+S3+S5 per chunk: value proj, offsets/attn,
                #      sampling weights, indices
                with tc.tile_pool(name=f"ps{li}", bufs=2, space="PSUM") as psp, \
                     tc.tile_pool(name=f"k1_{li}", bufs=2) as k1pool:
                    cur_kpool[0] = k1pool
                    for co, cw in _chunks(MMCH):
                        qs = slice(co, co + cw)
                        och = [T(grp="o") for _ in range(2)]
                        qb = [T(dtype=BF16, grp="q") for _ in range(2)]
                        for k in range(2):
                            nc.sync.dma_start(och[k][:, :cw], cur[k, :, qs])
                            pc = T(grp="o")
                            nc.sync.dma_start(pc[:, :cw], posT[k, :, qs])
                            nc.vector.tensor_tensor(pc[:, :cw], och[k][:, :cw],
                                                    pc[:, :cw], AL.add)
                            nc.vector.tensor_copy(qb[k][:, :cw], pc[:, :cw])
                        # value projection -> cc_in (DRAM)
                        for dt in range(2):
                            ps = psp.tile([128, 512], F32, tag="mm", name="mm")
                            ob = [T(dtype=BF16, grp="q") for _ in range(2)]
                            for k in range(2):
                                nc.vector.tensor_copy(ob[k][:, :cw], och[k][:, :cw])
                            for k in range(2):
                                nc.tensor.matmul(
                                    ps[:, :cw], Wval_t[k][:, dt * 128 : dt * 128 + 128],
                                    ob[k][:, :cw], start=(k == 0), stop=(k == 1))
                            vch = T(dtype=BF16, grp="v")
                            nc.scalar.activation(vch[:, :cw], ps[:, :cw], AF.Identity,
                                                 bias=bcol("bval", dt))
                            nc.sync.dma_start(cc_in[dt * 128 : dt * 128 + 128, qs],
                                              vch[:, :cw])

                        def proj128(wt, bcol):
                            ps = psp.tile([128, 512], F32, tag="mm", name="mm")
                            for k in range(2):
                                nc.tensor.matmul(ps[:, :cw], wt[k][:], qb[k][:, :cw],
                                                 start=(k == 0), stop=(k == 1))
                            o = T(grp="p")
                            nc.scalar.activation(o[:, :cw], ps[:, :cw], AF.Identity,
                                                 bias=bcol)
                            return o

                        offx = proj128(Woffx_t, bcol("boffx", 0))
                        offy = proj128(Woffy_t, bcol("boffy", 0))
                        psl = psp.tile([128, 512], F32, tag="mm", name="mm")
                        for k in range(2):
                            nc.tensor.matmul(psl[:, :cw], Wattn_t[k][:], qb[k][:, :cw],
                                             start=(k == 0), stop=(k == 1))
                        expt = T(grp="p")
                        nc.scalar.activation(expt[:, :cw], psl[:, :cw], AF.Exp,
                                             bias=bcol("battn", 0))
                        psd = psp.tile([8, 512], F32, tag="den", name="den")
                        nc.tensor.matmul(psd[:, :cw], bones_t[:], expt[:, :cw])
                        r8 = T([8, 512], grp="r")
                        nc.vector.reciprocal(r8[:, :cw], psd[:, :cw])
                        psr = psp.tile([128, 512], F32, tag="rep", name="rep")
                        nc.tensor.matmul(psr[:, :cw], sel16_t[:], r8[:, :cw])
                        attn = T(grp="p")
                        nc.vector.tensor_tensor(attn[:, :cw], expt[:, :cw],
                                                psr[:, :cw], AL.mult)

                        def floorfrac(off_sb, ref_dram):
                            x = T(grp="c")
                            rc = T(grp="c")
                            nc.sync.dma_start(rc[:, :cw], ref_dram[:, qs])
                            nc.vector.tensor_tensor(x[:, :cw], off_sb[:, :cw],
                                                    rc[:, :cw], AL.add)
                            r = T(grp="c")
                            nc.vector.tensor_scalar_add(r[:, :cw], x[:, :cw], MAGIC)
                            nc.vector.tensor_scalar_sub(r[:, :cw], r[:, :cw], MAGIC)
                            m = T(grp="c")
                            nc.vector.tensor_tensor(m[:, :cw], r[:, :cw], x[:, :cw],
                                                    AL.is_gt)
                            x0 = T(grp="f")
                            nc.vector.tensor_tensor(x0[:, :cw], r[:, :cw], m[:, :cw],
                                                    AL.subtract)
                            fx = T(grp="f")
                            nc.vector.tensor_tensor(fx[:, :cw], x[:, :cw], x0[:, :cw],
                                                    AL.subtract)
                            return x0, fx

                        x0, fx = floorfrac(offx, refx_d)
                        y0, fy = floorfrac(offy, refy_d)

                        def uv(c0, frac, hij):
                            a = T(grp="u")
                            nc.vector.tensor_scalar(a[:, :cw], c0[:, :cw], 0.0, None,
                                                    AL.is_ge)
                            b = T(grp="u")
                            nc.vector.tensor_scalar(b[:, :cw], c0[:, :cw],
                                                    col(ccol, hij), None, AL.is_le)
                            nc.vector.tensor_tensor(a[:, :cw], a[:, :cw], b[:, :cw],
                                                    AL.mult)
                            a1 = T(grp="u")
                            nc.vector.tensor_scalar(a1[:, :cw], c0[:, :cw], -1.0, None,
                                                    AL.is_ge)
                            b1 = T(grp="u")
                            nc.vector.tensor_scalar(b1[:, :cw], c0[:, :cw],
                                                    col(ccol, hij + 1), None, AL.is_le)
                            nc.vector.tensor_tensor(a1[:, :cw], a1[:, :cw], b1[:, :cw],
                                                    AL.mult)
                            omf = T(grp="w")
                            nc.vector.tensor_scalar(omf[:, :cw], frac[:, :cw], -1.0,
                                                    1.0, AL.mult, AL.add)
                            u0 = T(grp="w")
                            nc.vector.tensor_tensor(u0[:, :cw], omf[:, :cw], a[:, :cw],
                                                    AL.mult)
                            u1 = T(grp="w")
                            nc.vector.tensor_tensor(u1[:, :cw], frac[:, :cw],
                                                    a1[:, :cw], AL.mult)
                            return u0, u1

                        ux0, ux1 = uv(x0, fx, 0)
                        ty0, ty1 = uv(y0, fy, 2)
                        at0 = T(grp="w")
                        nc.vector.tensor_tensor(at0[:, :cw], attn[:, :cw], ty0[:, :cw],
                                                AL.mult)
                        at1 = T(grp="w")
                        nc.vector.tensor_tensor(at1[:, :cw], attn[:, :cw], ty1[:, :cw],
                                                AL.mult)
                        nc.vector.tensor_tensor(w4all[:, 0, qs], at0[:, :cw],
                                                ux0[:, :cw], AL.mult)
                        nc.vector.tensor_tensor(w4all[:, 1, qs], at0[:, :cw],
                                                ux1[:, :cw], AL.mult)
                        nc.vector.tensor_tensor(w4all[:, 2, qs], at1[:, :cw],
                                                ux0[:, :cw], AL.mult)
                        nc.vector.tensor_tensor(w4all[:, 3, qs], at1[:, :cw],
                                                ux1[:, :cw], AL.mult)
                        cx = T(grp="i")
                        nc.vector.tensor_scalar_max(cx[:, :cw], x0[:, :cw], -1.0)
                        nc.vector.tensor_scalar(cx[:, :cw], cx[:, :cw], col(ccol, 0),
                                                None, AL.min)
                        cy = T(grp="i")
                        nc.vector.tensor_scalar_max(cy[:, :cw], y0[:, :cw], -1.0)
                        nc.vector.tensor_scalar(cy[:, :cw], cy[:, :cw], col(ccol, 2),
                                                None, AL.min)
                        qi = T(grp="i")
                        nc.vector.tensor_scalar(qi[:, :cw], cy[:, :cw], col(ccol, 4),
                                                col(ccol, 5), AL.mult, AL.add)
                        nc.vector.tensor_tensor(qi[:, :cw], qi[:, :cw], cx[:, :cw],
                                                AL.add)
                        nc.vector.tensor_copy(idxT[:, qs], qi[:, :cw])

                # ---- exchange value halves
                tabpool_cm = tc.tile_pool(name=f"tab{li}", bufs=1)
                tabpool = tabpool_cm.__enter__()
                tab = tabpool.tile([128, TTOT * 4], U32, tag="quadtab", name="quadtab")
                tbv = tab[:].bitcast(BF16)
                vpool_cm = tc.tile_pool(name=f"vst{li}", bufs=1)
                vpool = vpool_cm.__enter__()
                vfull2 = vpool.tile([128, 2 * VW2], BF16, tag="vfull2", name="vfull2")
                for par in range(2):
                    nc.vector.memset(vfull2[:, par * VW2 : par * VW2 + VPAD], 0.0)
                    nc.vector.memset(vfull2[:, par * VW2 + VPAD + N : (par + 1) * VW2], 0.0)
                nc.gpsimd.collective_compute(
                    "AllGather", AL.bypass,
                    replica_groups=[[0, 1]] if sim2 else [[0, 1], [2, 3], [4, 5], [6, 7]],
                    ins=[cc_in[:].opt()], outs=[cc_out[:].opt()])
                for r in range(2):
                    for par in range(2):
                        nc.sync.dma_start(
                            vfull2[:, par * VW2 + VPAD + r * QH :
                                   par * VW2 + VPAD + (r + 1) * QH],
                            cc_out[r, par * 128 : par * 128 + 128, :])

                # ---- quad table build: entry t=(r,c) of level lv packs, for
                #      both dh parities, bf16 pairs (v[y,x],v[y,x+1]) at
                #      y in {r-1, r}, x = c-1.
                for lv in range(LVLS):
                    th, tw = TDIM[lv]
                    W_ = SHAPES[lv][1]
                    use_scalar = lv == 0
                    for par in range(2):
                        for y in range(2):
                            for j in range(2):
                                dst = bass.AP(
                                    tbv.tensor,
                                    tbv.offset + 8 * TSTART[lv] + par * 4 + y * 2 + j,
                                    [list(tbv.ap[0]), [8 * tw, th], [8, tw]])
                                src = bass.AP(
                                    vfull2[:].tensor,
                                    vfull2[:].offset + par * VW2 + VPAD
                                    + LSTART[lv] - W_ - 1 + y * W_ + j,
                                    [list(vfull2[:].ap[0]), [W_, th], [1, tw]])
                                if use_scalar:
                                    nc.scalar.copy(dst, src)
                                else:
                                    nc.vector.tensor_copy(dst, src)

                vpool_cm.__exit__(None, None, None)

                # ---- blend: per CW-query chunk, one d=4 gather + PE weight
                #      replication + DVE multiply/reduce
                with tc.tile_pool(name=f"psw{li}", bufs=1, space="PSUM") as pswp, \
                     tc.tile_pool(name=f"bp{li}", bufs=2) as bpool:
                    w4v = w4all[:].rearrange("p a b -> p (a b)")
                    for co, cwb in _chunks(BCH):
                        pw = pswp.tile([128, 16 * 4 * 64], F32, tag="pw", name="pw")
                        for lp in range(16):
                            mov = bass.AP(w4v.tensor, w4v.offset + co,
                                          [list(w4v.ap[0]), [1, cwb], [QH, 4]])
                            nc.tensor.matmul(pw[:, lp * 4 * cwb : lp * 4 * cwb + 4 * cwb],
                                             sel_t[:, lp, :], mov)
                        g = bpool.tile([128, 64 * 16 * 4], U32, tag="G", name="G")
                        nc.gpsimd.ap_gather(
                            g[:, : cwb * 16 * 4], tab[:, : TTOT * 4],
                            idxT[:, co : co + cwb],
                            channels=128, num_elems=TTOT, d=4, num_idxs=cwb * 16)
                        gv = g[:].bitcast(BF16)
                        pwv = pw[:]
                        ws = bass.AP(pwv.tensor, pwv.offset,
                                     [list(pwv.ap[0]), [4, cwb], [4 * cwb, 16], [1, 4]])
                        for par in range(2):
                            gs = bass.AP(gv.tensor, gv.offset + 4 * par,
                                         [list(gv.ap[0]), [128, cwb], [8, 16], [1, 4]])
                            nc.vector.tensor_tensor(gs, gs, ws, AL.mult)
                        mfv = msdaF[:]
                        for par in range(2):
                            src = bass.AP(gv.tensor, gv.offset + 4 * par,
                                          [list(gv.ap[0]), [128, cwb], [8, 16], [1, 4]])
                            dst = bass.AP(mfv.tensor, mfv.offset + 2 * co + par,
                                          [list(mfv.ap[0]), [2, cwb]])
                            nc.vector.tensor_reduce(dst, src, AX.XY, AL.add,
                                                    opt_input=False, opt_output=False)
                tabpool_cm.__exit__(None, None, None)

                # ---- W_out + residual + LN1 ; FFN + residual + LN2
                with tc.tile_pool(name=f"pso{li}", bufs=2, space="PSUM") as psp, \
                     tc.tile_pool(name=f"k2_{li}", bufs=2) as k2pool:
                    cur_kpool[0] = k2pool
                    for co, cw in _chunks(MMCH):
                        qs = slice(co, co + cw)

                        def layernorm(xin, gname, bename, dst0, dst1, outdram):
                            pss = psp.tile([1, 512], F32, tag="st1", bufs=1, name="st1")
                            for k in range(2):
                                nc.tensor.matmul(pss[:, :cw], ones128_t[:],
                                                 xin[k][:, :cw],
                                                 start=(k == 0), stop=(k == 1))
                            psq = psp.tile([1, 512], F32, tag="st2", bufs=1, name="st2")
                            for k in range(2):
                                xsq = T(grp="s")
                                nc.vector.tensor_tensor(xsq[:, :cw], xin[k][:, :cw],
                                                        xin[k][:, :cw], AL.mult)
                                nc.tensor.matmul(psq[:, :cw], ones128_t[:],
                                                 xsq[:, :cw],
                                                 start=(k == 0), stop=(k == 1))
                            mu = T([1, 512], grp="m")
                            nc.vector.tensor_scalar_mul(mu[:, :cw], pss[:, :cw],
                                                        1.0 / D)
                            var = T([1, 512], grp="m")
                            nc.vector.tensor_scalar_mul(var[:, :cw], psq[:, :cw],
                                                        1.0 / D)
                            mu2 = T([1, 512], grp="m")
                            nc.vector.tensor_tensor(mu2[:, :cw], mu[:, :cw],
                                                    mu[:, :cw], AL.mult)
                            nc.vector.tensor_tensor(var[:, :cw], var[:, :cw],
                                                    mu2[:, :cw], AL.subtract)
                            nc.vector.tensor_scalar_add(var[:, :cw], var[:, :cw], 1e-5)
                            rv_ = T([1, 512], grp="m")
                            nc.vector.reciprocal(rv_[:, :cw], var[:, :cw])
                            rstd = T([1, 512], grp="m")
                            nc.scalar.activation(rstd[:, :cw], rv_[:, :cw], AF.Sqrt)
                            psmu = psp.tile([128, 512], F32, tag="rpm", bufs=1, name="rpm")
                            nc.tensor.matmul(psmu[:, :cw], ones1x_t[:], mu[:, :cw])
                            psrs = psp.tile([128, 512], F32, tag="rps", bufs=1, name="rps")
                            nc.tensor.matmul(psrs[:, :cw], ones1x_t[:], rstd[:, :cw])
                            for k, dst in enumerate([dst0, dst1]):
                                xc = T(grp="s")
                                nc.vector.tensor_tensor(xc[:, :cw], xin[k][:, :cw],
                                                        psmu[:, :cw], AL.subtract)
                                nc.vector.tensor_tensor(xc[:, :cw], xc[:, :cw],
                                                        psrs[:, :cw], AL.mult)
                                nc.scalar.activation(dst[:, :cw], xc[:, :cw],
                                                     AF.Identity,
                                                     scale=bcol(gname, k),
                                                     bias=bcol(bename, k))
                                if outdram is not None:
                                    nc.sync.dma_start(outdram[k, :, qs], dst[:, :cw])

                        x1 = []
                        mb = cur_kpool[0].tile([128, 1024], BF16, tag="mb", name="mb")
                        nc.vector.tensor_copy(mb[:, : 2 * cw],
                                              msdaF[:, 2 * co : 2 * co + 2 * cw])
                        mbv = mb[:]
                        for dt in range(2):
                            ps = psp.tile([128, 512], F32, tag="mm", name="mm")
                            for k in range(2):
                                movk = bass.AP(mbv.tensor, mbv.offset + k,
                                               [list(mbv.ap[0]), [2, cw]])
                                nc.tensor.matmul(
                                    ps[:, :cw],
                                    Wout_t[k][:, dt * 128 : dt * 128 + 128],
                                    movk, start=(k == 0), stop=(k == 1))
                            t0 = T(grp="x")
                            nc.scalar.activation(t0[:, :cw], ps[:, :cw], AF.Identity,
                                                 bias=bcol("bout", dt))
                            och = T(grp="x")
                            nc.sync.dma_start(och[:, :cw], cur[dt, :, qs])
                            nc.vector.tensor_tensor(t0[:, :cw], t0[:, :cw],
                                                    och[:, :cw], AL.add)
                            x1.append(t0)
                        ln1 = [T(grp="l") for _ in range(2)]
                        layernorm(x1, "g1", "be1", ln1[0], ln1[1], None)
                        ln1b = [T(dtype=BF16, grp="lb") for _ in range(2)]
                        for dt in range(2):
                            nc.vector.tensor_copy(ln1b[dt][:, :cw], ln1[dt][:, :cw])
                        hidb = [T(dtype=BF16, grp=f"h{m}") for m in range(8)]
                        for m in range(8):
                            ph = psp.tile([128, 512], F32, tag="mm", name="mm")
                            for k in range(2):
                                nc.tensor.matmul(
                                    ph[:, :cw], W1_t[k][:, m * 128 : m * 128 + 128],
                                    ln1b[k][:, :cw], start=(k == 0), stop=(k == 1))
                            nc.scalar.activation(hidb[m][:, :cw], ph[:, :cw], AF.Relu,
                                                 bias=bcol("bl1", m))
                        x2 = []
                        for dt in range(2):
                            ps = psp.tile([128, 512], F32, tag="mm", name="mm")
                            for k in range(8):
                                nc.tensor.matmul(
                                    ps[:, :cw],
                                    W2_t[k][:, dt * 128 : dt * 128 + 128],
                                    hidb[k][:, :cw], start=(k == 0), stop=(k == 7))
                            t0 = T(grp="x")
                            nc.scalar.activation(t0[:, :cw], ps[:, :cw], AF.Identity,
                                                 bias=bcol("bl2", dt))
                            nc.vector.tensor_tensor(t0[:, :cw], t0[:, :cw],
                                                    ln1[dt][:, :cw], AL.add)
                            x2.append(t0)
                        no = [T(grp="n") for _ in range(2)]
                        layernorm(x2, "g2", "be2", no[0], no[1], nxt)
                cur = nxt

    nc.compile()
    return nc


# ---------------- host side ----------------

def _host_prep(inputs, n_layers=NLAYERS):
    f32 = np.float32
    bf16 = ml_dtypes.bfloat16
    L = n_layers
    inputs = dict(inputs)
    for nm in ["W_off", "b_off", "W_attn", "b_attn", "W_val", "b_val",
               "W_out", "b_out", "W1", "bl1", "W2", "bl2",
               "g1", "be1", "g2", "be2"]:
        inputs[nm] = np.asarray(inputs[nm])[:L]
    vr = np.asarray(inputs["valid_ratios"], f32)
    refs = []
    for lvl, (H_, W_) in enumerate(SHAPES):
        ry, rx = np.meshgrid(
            np.linspace(0.5, H_ - 0.5, H_, dtype=f32),
            np.linspace(0.5, W_ - 0.5, W_, dtype=f32), indexing="ij")
        ry = ry.reshape(-1)[None] / (vr[:, None, lvl, 1] * H_)
        rx = rx.reshape(-1)[None] / (vr[:, None, lvl, 0] * W_)
        refs.append(np.stack([rx, ry], -1))
    ref = np.concatenate(refs, 1)
    ref = ref[:, :, None] * vr[:, None]                    # [B, N, LVLS, 2]

    Wd = np.array([w for h, w in SHAPES], f32)
    Hd = np.array([h for h, w in SHAPES], f32)
    lrow = np.tile(np.repeat(np.arange(LVLS), PTS), H)     # [128]
    refx_all = ref[:, :, :, 0] * Wd[None, None] - 0.5
    refy_all = ref[:, :, :, 1] * Hd[None, None] - 0.5

    W_off = np.asarray(inputs["W_off"], f32).reshape(L, D, H, LVLS, PTS, 2)
    b_off = np.asarray(inputs["b_off"], f32).reshape(L, H, LVLS, PTS, 2)
    Woffx = W_off[..., 0].reshape(L, D, 128)
    Woffy = W_off[..., 1].reshape(L, D, 128)

    # feature permutation: slot (par, p) with p=(h, dp) holds original
    # feature h*32 + 2*dp + par
    newcol = np.empty(256, np.int64)
    for c in range(256):
        par, p = c // 128, c % 128
        h_, dp = p // 16, p % 16
        newcol[c] = h_ * 32 + 2 * dp + par

    def kt(w, nk):
        return np.ascontiguousarray(
            np.asarray(w, f32).reshape(L, nk, 128, -1)).astype(bf16)

    def bc(v, w):
        return np.ascontiguousarray(
            np.asarray(v, f32).reshape(L, w, 128).transpose(0, 2, 1))

    Wval_p = np.asarray(inputs["W_val"], f32)[:, :, newcol]
    bval_p = np.asarray(inputs["b_val"], f32)[:, newcol]
    Wout_p = np.asarray(inputs["W_out"], f32)[:, newcol, :]

    shared = {
        "Woffx": kt(Woffx, 2), "Woffy": kt(Woffy, 2),
        "Wattn": kt(inputs["W_attn"], 2), "Wval": kt(Wval_p, 2),
        "Wout": kt(Wout_p, 2), "W1": kt(inputs["W1"], 2),
        "W2": kt(inputs["W2"], 8),
    }
    bias_all = np.zeros((L, 128, 25), f32)
    bias_all[:, :, 0] = b_off[..., 0].reshape(L, 128)
    bias_all[:, :, 1] = b_off[..., 1].reshape(L, 128)
    bias_all[:, :, 2] = np.asarray(inputs["b_attn"], f32).reshape(L, 128)
    bias_all[:, :, 3:5] = bc(bval_p, 2)
    bias_all[:, :, 5:7] = bc(inputs["b_out"], 2)
    bias_all[:, :, 7:15] = bc(inputs["bl1"], 8)
    bias_all[:, :, 15:17] = bc(inputs["bl2"], 2)
    bias_all[:, :, 17:19] = bc(inputs["g1"], 2)
    bias_all[:, :, 19:21] = bc(inputs["be1"], 2)
    bias_all[:, :, 21:23] = bc(inputs["g2"], 2)
    bias_all[:, :, 23:25] = bc(inputs["be2"], 2)
    shared["bias_all"] = bias_all
    # selector: out partition (h, dp) <- w4all row (h, lp), for each lp
    sel = np.zeros((128, 16, 128), f32)
    for h in range(H):
        for lp in range(16):
            sel[h * 16 + lp, lp, h * 16 : h * 16 + 16] = 1.0
    shared["sel"] = sel.astype(bf16)
    bones = np.zeros((128, 8), f32)
    for h in range(H):
        bones[h * 16 : h * 16 + 16, h] = 1.0
    shared["bones"] = bones
    sel16 = np.zeros((8, 128), f32)
    for h in range(H):
        sel16[h, h * 16 : h * 16 + 16] = 1.0
    shared["sel16"] = sel16
    shared["ones128"] = np.ones((128, 1), f32)
    shared["ones1x"] = np.ones((1, 128), f32)
    ccol = np.zeros((128, 8), f32)
    for p in range(128):
        lv = lrow[p]
        ccol[p, 0] = Wd[lv] - 1
        ccol[p, 1] = Wd[lv] - 2
        ccol[p, 2] = Hd[lv] - 1
        ccol[p, 3] = Hd[lv] - 2
        ccol[p, 4] = Wd[lv] + 1
        ccol[p, 5] = TSTART[lv] + Wd[lv] + 2
    shared["ccol"] = ccol

    src = np.asarray(inputs["src"], f32)
    pos = np.asarray(inputs["pos"], f32)
    per_core = []
    for c in range(8):
        b, hf = c // 2, c % 2
        qs = slice(hf * QH, (hf + 1) * QH)
        m = dict(shared)
        m["x0T"] = np.ascontiguousarray(src[b, qs].T).reshape(2, 128, QH)
        m["posT"] = np.ascontiguousarray(pos[b, qs].T).reshape(2, 128, QH)
        m["refx"] = np.ascontiguousarray(refx_all[b, qs][:, lrow].T)
        m["refy"] = np.ascontiguousarray(refy_all[b, qs][:, lrow].T)
        per_core.append(m)
    return per_core


_NC_CACHE = {}


def kernel(**inputs):
    if NLAYERS not in _NC_CACHE:
        _NC_CACHE[NLAYERS] = build_module(NLAYERS)
    nc = _NC_CACHE[NLAYERS]
    in_maps = _host_prep(inputs, NLAYERS)
    res = run_bass_kernel_spmd(nc, in_maps, core_ids=list(range(8)))
    out = np.empty((B, N, D), np.float32)
    for c in range(8):
        b, hf = c // 2, c % 2
        o = res.results[c]["outT"]
        out[b, hf * QH : (hf + 1) * QH, :] = o.reshape(256, QH).T
    return out


if __name__ == "__main__":
    import reference
    inp = {k: np.asarray(v) for k, v in reference.setup_inputs().items()}
    got = kernel(**inp)
    print("kernel output:", got.shape, got.dtype)
